# revision 21
# baseline (speedup 1.0000x reference)
"""Physics-informed loss kernel for Trainium2 (8 NeuronCores, data parallel).

Input: gen_output [128, 3, 256, 256] f32. Output: scalar f32 loss.

Layout per core (16 images): partition p = slab*16 + image, 8 row-slabs of 32
rows each; free dim = 34 rows x 256 cols (32-row slab + 2 halo rows) so every
stencil shift is a free-dim AP offset. Per-partition partial sums are DMA'd
out and combined on host.
"""

import sys

import numpy as np

sys.path.insert(0, "/opt/trn_rl_repo")

from concourse.bass import Bass
from concourse import mybir
from concourse.bass_utils import run_bass_kernel_spmd

N_CORES = 8
BPC = 16  # images per core
H = W = 256
NSLAB = 8
GRID = 8192        # 32 rows * 256
FREE = 8712       # 34 rows * 256 + 8 pad
F33 = 8456        # 33 rows * 256 + 8 pad
CH = 4096         # chunk = 16 rows

DX = 0.01
U0 = 1.0
LC, LB = 0.3, 0.2
NCONT = 254.0 * 254.0
NPOIS = 254.0 * 253.0
NACC = 212

_CACHE = {}


class _Seq:
    """Serialize one engine: every op waits for the previous op's completion
    via a self-semaphore (deep pipelines => same-engine RAW needs sync),
    keeping each instruction at <=1 embedded sync wait for walrus codegen."""

    def __init__(self, eng, sem):
        self.eng = eng
        self.sem = sem
        self.n = 0

    def __getattr__(self, name):
        m = getattr(self.eng, name)

        def call(*a, **k):
            if self.n:
                self.eng.wait_ge(self.sem, self.n)
            m(*a, **k).then_inc(self.sem, 1)
            self.n += 1

        return call


def _build_program():
    AL = mybir.AluOpType
    AX = mybir.AxisListType
    f32 = mybir.dt.float32

    nc = Bass("TRN2", target_bir_lowering=False, debug=False, num_devices=N_CORES)
    x = nc.dram_tensor("x", [BPC, 3, H, W], f32, kind="ExternalInput").ap()
    acc_d = nc.dram_tensor("acc", [128, NACC], f32, kind="ExternalOutput").ap()

    with (
        nc.Block() as block,
        nc.semaphore("dma_sem") as dma_sem,
        nc.semaphore("vs") as vs,
        nc.sbuf_tensor("big", [128, 3 * FREE], f32) as big,
        nc.sbuf_tensor("P4", [128, F33], f32) as P4,
        nc.sbuf_tensor("sq", [128, F33], f32) as sq,
        nc.sbuf_tensor("dc", [128, CH], f32) as dc,
        nc.sbuf_tensor("ac", [128, NACC], f32) as ac,
    ):
        CU, CV, CP = 0, FREE, 2 * FREE
        bv = big[:, :].rearrange("p (c f) -> p c f", c=3)

        nops = []

        @block.vector
        def _(vector):
            V = _Seq(vector, vs)
            V.memset(ac[:, :], 0.0)
            V.memset(bv[:, :, 8704:FREE], 0.0)
            vector.wait_ge(dma_sem, 48)

            # ---- continuity: C = u@257 - u@256 + v@257 - v@1 (=(dudx+dvdy)*DX)
            V.scalar_tensor_tensor(
                sq[:, 0:GRID], big[:, 257:257 + GRID], 1.0,
                big[:, 256:256 + GRID], AL.mult, AL.subtract)
            V.scalar_tensor_tensor(
                sq[:, 0:GRID], big[:, CV + 257:CV + 257 + GRID], 1.0,
                sq[:, 0:GRID], AL.mult, AL.add)
            V.scalar_tensor_tensor(
                sq[:, 0:GRID], big[:, CV + 1:CV + 1 + GRID], -1.0,
                sq[:, 0:GRID], AL.mult, AL.add)
            c3 = sq[:, 0:GRID].rearrange("p (r w) -> p r w", w=W)
            V.tensor_reduce(
                ac[:, 0:1], c3[:, 0:30, 0:254], AX.XY, AL.add,
                apply_absolute_value=True)
            V.tensor_reduce(
                ac[:, 1:2], c3[:, 30:32, 0:254], AX.XY, AL.add,
                apply_absolute_value=True)

            # ---- shared products: P4 = (v@0+v@1)*(u@0+u@256); squ4 = (u@0+u@1)^2
            V.scalar_tensor_tensor(
                sq[:, 0:F33], big[:, 0:F33], 1.0, big[:, 256:256 + F33],
                AL.mult, AL.add)
            V.scalar_tensor_tensor(
                P4[:, 0:F33], big[:, CV:CV + F33], 1.0,
                big[:, CV + 1:CV + 1 + F33], AL.mult, AL.add)
            V.scalar_tensor_tensor(
                P4[:, 0:F33], P4[:, 0:F33], 1.0, sq[:, 0:F33],
                AL.mult, AL.mult)
            V.scalar_tensor_tensor(
                sq[:, 0:F33], big[:, 0:F33], 1.0, big[:, 1:1 + F33],
                AL.mult, AL.add)
            V.scalar_tensor_tensor(
                sq[:, 0:F33], sq[:, 0:F33], 1.0, sq[:, 0:F33],
                AL.mult, AL.mult)                                         # squ4

            # ---- D = -4*DX*dudt, accumulated per 16-row chunk
            du_taps = [(P4, 0, 257, 1.0), (P4, 0, 1, -1.0),
                       (big, CU, 258, -0.4), (big, CU, 256, -0.4),
                       (big, CU, 513, -0.4), (big, CU, 1, -0.4),
                       (big, CU, 257, 1.6),
                       (big, CP, 258, 4.0), (big, CP, 257, -4.0)]
            for k in range(2):
                b0 = k * CH
                V.scalar_tensor_tensor(
                    dc[:, 0:CH], sq[:, b0 + 257:b0 + 257 + CH], 1.0,
                    sq[:, b0 + 256:b0 + 256 + CH], AL.mult, AL.subtract)
                for tbuf, base, off, coef in du_taps:
                    o = base + b0 + off
                    V.scalar_tensor_tensor(
                        dc[:, 0:CH], tbuf[:, o:o + CH], coef,
                        dc[:, 0:CH], AL.mult, AL.add)
                d3 = dc[:, 0:CH].rearrange("p (r w) -> p r w", w=W)
                if k == 0:
                    V.tensor_reduce(
                        ac[:, 2:3], d3[:, 0:16, 0:253], AX.XY, AL.add,
                        apply_absolute_value=True)
                else:
                    V.tensor_reduce(
                        ac[:, 3:4], d3[:, 0:14, 0:253], AX.XY, AL.add,
                        apply_absolute_value=True)
                    V.tensor_reduce(
                        ac[:, 4:5], d3[:, 14:16, 0:253], AX.XY, AL.add,
                        apply_absolute_value=True)

            # ---- sqv4 = (v@0+v@256)^2
            V.scalar_tensor_tensor(
                sq[:, 0:F33], big[:, CV:CV + F33], 1.0,
                big[:, CV + 256:CV + 256 + F33], AL.mult, AL.add)
            V.scalar_tensor_tensor(
                sq[:, 0:F33], sq[:, 0:F33], 1.0, sq[:, 0:F33],
                AL.mult, AL.mult)                                         # sqv4

            # ---- D' = -4*DX*dvdt
            dv_taps = [(sq, 0, 257, 1.0), (sq, 0, 1, -1.0),
                       (big, CV, 258, -0.4), (big, CV, 256, -0.4),
                       (big, CV, 513, -0.4), (big, CV, 1, -0.4),
                       (big, CV, 257, 1.6),
                       (big, CP, 513, 4.0), (big, CP, 257, -4.0)]
            for k in range(2):
                b0 = k * CH
                V.scalar_tensor_tensor(
                    dc[:, 0:CH], P4[:, b0 + 257:b0 + 257 + CH], 1.0,
                    P4[:, b0 + 256:b0 + 256 + CH], AL.mult, AL.subtract)
                for tbuf, base, off, coef in dv_taps:
                    o = base + b0 + off
                    V.scalar_tensor_tensor(
                        dc[:, 0:CH], tbuf[:, o:o + CH], coef,
                        dc[:, 0:CH], AL.mult, AL.add)
                d3 = dc[:, 0:CH].rearrange("p (r w) -> p r w", w=W)
                if k == 0:
                    V.tensor_reduce(
                        ac[:, 5:6], d3[:, 0:16, 0:254], AX.XY, AL.add,
                        apply_absolute_value=True)
                else:
                    V.tensor_reduce(
                        ac[:, 6:7], d3[:, 0:13, 0:254], AX.XY, AL.add,
                        apply_absolute_value=True)
                    V.tensor_reduce(
                        ac[:, 7:8], d3[:, 13:16, 0:254], AX.XY, AL.add,
                        apply_absolute_value=True)

            # ---- boundary terms (plain sums; abs + cross-slab combine on host)
            u3 = big[:, CU:CU + 8704].rearrange("p (r w) -> p r w", w=W)
            v3 = big[:, CV:CV + 8704].rearrange("p (r w) -> p r w", w=W)
            p3 = big[:, CP:CP + 8704].rearrange("p (r w) -> p r w", w=W)
            # y0: ac8 = sum(u row0+row1) + sum(v row0 + p row0); ac9=ac10=0
            V.scalar_tensor_tensor(
                dc[:, 0:253], big[:, CU + 1:CU + 254], 1.0,
                big[:, CU + 257:CU + 510], AL.mult, AL.add)
            V.scalar_tensor_tensor(
                dc[:, 253:507], big[:, CV + 1:CV + 255], 1.0,
                big[:, CP + 1:CP + 255], AL.mult, AL.add)
            V.tensor_reduce(ac[:, 8:9], dc[:, 0:507], AX.X, AL.add)
            # yL: ac11 = sum(u row30+row31); ac12 = sum(v row31 + p row31)
            V.scalar_tensor_tensor(
                dc[:, 507:760], big[:, CU + 7681:CU + 7934], 1.0,
                big[:, CU + 7937:CU + 8190], AL.mult, AL.add)
            V.scalar_tensor_tensor(
                dc[:, 760:1014], big[:, CV + 7937:CV + 8191], 1.0,
                big[:, CP + 7937:CP + 8191], AL.mult, AL.add)
            V.tensor_reduce(ac[:, 11:12], dc[:, 507:760], AX.X, AL.add)
            V.tensor_reduce(ac[:, 12:13], dc[:, 760:1014], AX.X, AL.add)
            # per-row column strips via stt adds (host applies row masks);
            # u/p blocks share a mask so they are pre-summed; ac[148:212]=0
            V.scalar_tensor_tensor(
                ac[:, 20:52], v3[:, 0:32, 0], 1.0, v3[:, 0:32, 1],
                AL.mult, AL.add)
            V.scalar_tensor_tensor(
                ac[:, 52:84], v3[:, 0:32, 254], 1.0, v3[:, 0:32, 255],
                AL.mult, AL.add)
            V.scalar_tensor_tensor(
                ac[:, 84:116], u3[:, 0:32, 0], 1.0, p3[:, 0:32, 0],
                AL.mult, AL.add)
            V.scalar_tensor_tensor(
                ac[:, 116:148], u3[:, 0:32, 255], 1.0, p3[:, 0:32, 255],
                AL.mult, AL.add)
            nops.append(V.n)

        @block.sync
        def _(sync):
            # 3 input DMAs; each +16 on completion
            xa = x.rearrange("m c (s r) w -> s m c (r w)", r=32)
            sync.dma_start(out=bv[:, :, 0:GRID], in_=xa).then_inc(dma_sem, 16)
            xh = x[:, :, 32:256, :].rearrange("m c (s r) w -> s m c (r w)", r=32)
            sync.dma_start(
                out=bv[0:112, :, GRID:8704], in_=xh[:, :, :, 0:512]
            ).then_inc(dma_sem, 16)
            # slab 7 halo: dummy rows, excluded by host masks
            sync.dma_start(
                out=bv[112:128, :, GRID:8704], in_=x[:, :, 0:2, :]
            ).then_inc(dma_sem, 16)
            sync.wait_ge(vs, nops[0])
            sync.dma_start(out=acc_d, in_=ac[:, :]).then_inc(dma_sem, 16)
            sync.wait_ge(dma_sem, 64)

    return nc


def _get_nc():
    if "nc" not in _CACHE:
        _CACHE["nc"] = _build_program()
    return _CACHE["nc"]


def combine(accs):
    """Host-side combine of per-core [128, 20] partial-sum tensors."""
    cont_sum = 0.0
    pois_sum = 0.0
    bc_total = 0.0

    def msum(r, last):
        # r: [8 slabs, 16, 32 rows]; global rows 1..253 (last=30) or 1..254 (last=31)
        return (r[0, :, 1:].sum(-1) + r[1:7, :, :].sum((0, 2))
                + r[7, :, 0:last].sum(-1))

    for a in accs:
        A = np.asarray(a, dtype=np.float64).reshape(NSLAB, BPC, NACC)
        s_c = A[:, :, 0].sum(0) + A[0:7, :, 1].sum(0)
        s_d = A[:, :, 2].sum(0) + A[:, :, 3].sum(0) + A[0:7, :, 4].sum(0)
        s_dp = A[:, :, 5].sum(0) + A[:, :, 6].sum(0) + A[0:7, :, 7].sum(0)
        cont_sum += (s_c / (NCONT * DX)).sum()
        pois_sum += ((s_d + s_dp) / (NPOIS * 4.0 * DX)).sum()
        y0 = A[0, :, 8] + A[0, :, 9] + A[0, :, 10]
        yl = 2.0 * U0 * 253.0 - A[7, :, 11] + A[7, :, 12] + A[7, :, 13]
        x0 = (msum(A[:, :, 20:52], 30) + msum(A[:, :, 84:116], 31)
              + msum(A[:, :, 148:180], 31))
        xl = (msum(A[:, :, 52:84], 30) + msum(A[:, :, 116:148], 31)
              + msum(A[:, :, 180:212], 31))
        bc_total += (np.abs(y0) + np.abs(yl) + np.abs(x0) + np.abs(xl)).sum()
    nb = float(N_CORES * BPC)
    loss = LC * cont_sum / nb + (1.0 - LC - LB) * pois_sum / nb + LB * bc_total
    return np.array(loss, dtype=np.float32)


def run(gen_output, trace=False):
    gen_output = np.asarray(gen_output, dtype=np.float32)
    nc = _get_nc()
    in_maps = [
        {"x": np.ascontiguousarray(gen_output[c * BPC:(c + 1) * BPC])}
        for c in range(N_CORES)
    ]
    res = run_bass_kernel_spmd(nc, in_maps, core_ids=list(range(N_CORES)), trace=trace)
    _CACHE["last"] = res
    return combine([res.results[c]["acc"] for c in range(N_CORES)])


def kernel(gen_output):
    return run(gen_output)


# revision 31
# speedup vs baseline: 1.1754x; 1.1754x over previous
"""Physics-informed loss kernel for Trainium2 (8 NeuronCores, data parallel).

Input: gen_output [128, 3, 256, 256] f32. Output: scalar f32 loss.

Layout per core (16 images): partition p = slab*16 + image, 8 row-slabs of 32
rows each; free dim = 34 rows x 256 cols (32-row slab + 2 halo rows) so every
stencil shift is a free-dim AP offset. Wide elementwise ops are range-split
between DVE (~2/3) and Pool (~1/3, runs at 0.42x roofline); squares and
abs-sum reduces run on ACT via activation(func, accum_out). Per-partition
partial sums are DMA'd out and combined on host.
"""

import sys

import numpy as np

sys.path.insert(0, "/opt/trn_rl_repo")

from concourse.bass import Bass
from concourse import mybir
from concourse.bass_utils import run_bass_kernel_spmd

N_CORES = 8
BPC = 16  # images per core
H = W = 256
NSLAB = 8
GRID = 8192        # 32 rows * 256
FREE = 8712       # 34 rows * 256 + 8 pad
F33 = 8456        # 33 rows * 256 + 8 pad
CH = 4096         # chunk = 16 rows

DX = 0.01
U0 = 1.0
LC, LB = 0.3, 0.2
NCONT = 254.0 * 254.0
NPOIS = 254.0 * 253.0
NACC = 212
POOL_ON = False   # V3 ISA rejects TensorScalarPtr on Pool engine
SPLIT = 1.0       # DVE share of each wide op (rest on Pool)

_CACHE = {}


class _Seq:
    """Serialize one engine: every op waits for the previous op's completion
    via a self-semaphore (deep pipelines => same-engine RAW needs sync),
    keeping each instruction at <=1 embedded sync wait for walrus codegen."""

    def __init__(self, eng, sem):
        self.eng = eng
        self.sem = sem
        self.n = 0

    def wait(self, sem, n):
        self.eng.wait_ge(sem, n)

    def mark(self, _name):
        pass

    def __getattr__(self, name):
        m = getattr(self.eng, name)

        def call(*a, **k):
            if self.n:
                self.eng.wait_ge(self.sem, self.n)
            m(*a, **k).then_inc(self.sem, 1)
            self.n += 1

        return call


class _Count:
    """Pass-0 stand-in: counts ops and records milestones, emits nothing."""

    def __init__(self, M, suffix):
        self.M = M
        self.suffix = suffix
        self.n = 0

    def wait(self, sem, n):
        pass

    def mark(self, name):
        self.M[name + self.suffix] = self.n

    def __getattr__(self, name):
        def call(*a, **k):
            self.n += 1

        return call


def _build_program():
    AL = mybir.AluOpType
    AF = mybir.ActivationFunctionType
    f32 = mybir.dt.float32

    nc = Bass("TRN2", target_bir_lowering=False, debug=False, num_devices=N_CORES)
    x = nc.dram_tensor("x", [BPC, 3, H, W], f32, kind="ExternalInput").ap()
    acc_d = nc.dram_tensor("acc", [128, NACC], f32, kind="ExternalOutput").ap()

    # ACT op count thresholds (fixed op order on ACT)
    A_CONT, A_SQ4, A_DK0, A_DK1, A_SQV, A_DPK0, A_DPK1 = 2, 3, 4, 6, 7, 8, 10

    with (
        nc.Block() as block,
        nc.semaphore("su") as su,
        nc.semaphore("sv") as sv,
        nc.semaphore("sp") as sp,
        nc.semaphore("so") as so,
        nc.semaphore("vs") as vs,
        nc.semaphore("ps") as ps,
        nc.semaphore("qs") as qs,
        nc.sbuf_tensor("big", [128, 3 * FREE], f32) as big,
        nc.sbuf_tensor("P4", [128, F33], f32) as P4,
        nc.sbuf_tensor("sq", [128, F33], f32) as sq,
        nc.sbuf_tensor("dc", [128, CH], f32) as dc,
        nc.sbuf_tensor("ac", [128, NACC], f32) as ac,
        nc.sbuf_tensor("zap", [128, 1], f32) as zap,
    ):
        CU, CV, CP = 0, FREE, 2 * FREE
        bv = big[:, :].rearrange("p (c f) -> p c f", c=3)
        M = {}

        du_taps = [(P4, 0, 257, 1.0), (P4, 0, 1, -1.0),
                   (big, CU, 258, -0.4), (big, CU, 256, -0.4),
                   (big, CU, 513, -0.4), (big, CU, 1, -0.4),
                   (big, CU, 257, 1.6),
                   (big, CP, 258, 4.0), (big, CP, 257, -4.0)]
        dv_taps = [(sq, 0, 257, 1.0), (sq, 0, 1, -1.0),
                   (big, CV, 258, -0.4), (big, CV, 256, -0.4),
                   (big, CV, 513, -0.4), (big, CV, 1, -0.4),
                   (big, CV, 257, 1.6),
                   (big, CP, 513, 4.0), (big, CP, 257, -4.0)]

        def body(S, isV):
            """Shared DVE/Pool program; wide ops sliced by engine share."""
            sfx_other = "_p" if isV else "_v"
            sem_other = ps if isV else vs

            def rng(n):
                if not POOL_ON:
                    return (0, n) if isV else (0, 0)
                mid = int(n * SPLIT) & ~3
                return (0, mid) if isV else (mid, n)

            def stt(out_buf, oo, b0_, o0, sc, b1_, o1, op0, op1, n):
                a, b = rng(n)
                if a == b:
                    return
                S.scalar_tensor_tensor(
                    out_buf[:, oo + a:oo + b], b0_[:, o0 + a:o0 + b], sc,
                    b1_[:, o1 + a:o1 + b], op0, op1)

            if isV:
                S.memset(ac[:, :], 0.0)
                S.memset(bv[:, :, 8704:FREE], 0.0)
                S.memset(zap[:, :], 0.0)
            else:
                S.wait(vs, 3)
            S.wait(su, 48)
            S.wait(sv, 48)

            # continuity C = u@257 - u@256 + v@257 - v@1, into sq[0:GRID]
            stt(sq, 0, big, 257, 1.0, big, 256, AL.mult, AL.subtract, GRID)
            stt(sq, 0, big, CV + 257, 1.0, sq, 0, AL.mult, AL.add, GRID)
            stt(sq, 0, big, CV + 1, -1.0, sq, 0, AL.mult, AL.add, GRID)
            S.mark("cont3")

            # P4 = (v@0+v@1)*(u@0+u@256) via dc-free staging in P4/sq? use P4
            stt(P4, 0, big, CV, 1.0, big, CV + 1, AL.mult, AL.add, F33)
            # stage (u@0+u@256) into dc? no: multiply P4 by sum via two steps:
            # tmp reuse: dcq = u@0+u@256 must live full F33 -> use dc? dc is
            # CH=4096 only. Stage into P4 then multiply in place with second
            # sum built in... use sq after cont reduces? cont reduce is on ACT
            # and reads sq -> wait qs>=A_CONT before overwriting sq.
            S.wait(qs, A_CONT)
            stt(sq, 0, big, CU, 1.0, big, CU + 256, AL.mult, AL.add, F33)
            stt(P4, 0, P4, 0, 1.0, sq, 0, AL.mult, AL.mult, F33)
            S.mark("p4")

            # squ4 = (u@0+u@1)^2 : add here, square on ACT
            stt(sq, 0, big, CU, 1.0, big, CU + 1, AL.mult, AL.add, F33)
            S.mark("sq4")

            # D chunks
            for k in range(2):
                b0 = k * CH
                S.wait(qs, A_SQ4 if k == 0 else A_DK0)
                stt(dc, 0, sq, b0 + 257, 1.0, sq, b0 + 256,
                    AL.mult, AL.subtract, CH)
                for ti, (tbuf, base, off, coef) in enumerate(du_taps):
                    if k == 0 and ti == 7:
                        S.wait(sp, 48)
                    stt(dc, 0, tbuf, base + b0 + off, coef, dc, 0,
                        AL.mult, AL.add, CH)
                S.mark("dk%d" % k)

            # sqv4 = (v@0+v@256)^2 : barrier on other engine's D-k1 init
            if POOL_ON:
                S.wait(sem_other, M.get("dk0" + sfx_other, 0) + 1)
            stt(sq, 0, big, CV, 1.0, big, CV + 256, AL.mult, AL.add, F33)
            S.mark("sqv")

            # D' chunks
            for k in range(2):
                b0 = k * CH
                S.wait(qs, A_SQV if k == 0 else A_DPK0)
                stt(dc, 0, P4, b0 + 257, 1.0, P4, b0 + 256,
                    AL.mult, AL.subtract, CH)
                for tbuf, base, off, coef in dv_taps:
                    stt(dc, 0, tbuf, base + b0 + off, coef, dc, 0,
                        AL.mult, AL.add, CH)
                S.mark("dpk%d" % k)

            if isV:
                # boundary terms, staged in P4 (dead after D' k1 inits)
                if POOL_ON:
                    S.wait(ps, M.get("dpk0_p", 0) + 1)
                AX = mybir.AxisListType
                u3 = big[:, CU:CU + 8704].rearrange("p (r w) -> p r w", w=W)
                v3 = big[:, CV:CV + 8704].rearrange("p (r w) -> p r w", w=W)
                p3 = big[:, CP:CP + 8704].rearrange("p (r w) -> p r w", w=W)
                S.scalar_tensor_tensor(
                    P4[:, 0:253], big[:, CU + 1:CU + 254], 1.0,
                    big[:, CU + 257:CU + 510], AL.mult, AL.add)
                S.scalar_tensor_tensor(
                    P4[:, 253:507], big[:, CV + 1:CV + 255], 1.0,
                    big[:, CP + 1:CP + 255], AL.mult, AL.add)
                S.tensor_reduce(ac[:, 8:9], P4[:, 0:507], AX.X, AL.add)
                S.scalar_tensor_tensor(
                    P4[:, 507:760], big[:, CU + 7681:CU + 7934], 1.0,
                    big[:, CU + 7937:CU + 8190], AL.mult, AL.add)
                S.scalar_tensor_tensor(
                    P4[:, 760:1014], big[:, CV + 7937:CV + 8191], 1.0,
                    big[:, CP + 7937:CP + 8191], AL.mult, AL.add)
                S.tensor_reduce(ac[:, 11:12], P4[:, 507:760], AX.X, AL.add)
                S.tensor_reduce(ac[:, 12:13], P4[:, 760:1014], AX.X, AL.add)
                S.scalar_tensor_tensor(
                    ac[:, 20:52], v3[:, 0:32, 0], 1.0, v3[:, 0:32, 1],
                    AL.mult, AL.add)
                S.scalar_tensor_tensor(
                    ac[:, 52:84], v3[:, 0:32, 254], 1.0, v3[:, 0:32, 255],
                    AL.mult, AL.add)
                S.scalar_tensor_tensor(
                    ac[:, 84:116], u3[:, 0:32, 0], 1.0, p3[:, 0:32, 0],
                    AL.mult, AL.add)
                S.scalar_tensor_tensor(
                    ac[:, 116:148], u3[:, 0:32, 255], 1.0, p3[:, 0:32, 255],
                    AL.mult, AL.add)
            S.mark("end")

        # pass 0: op counts / milestones for cross-engine waits
        body(_Count(M, "_v"), True)
        if POOL_ON:
            body(_Count(M, "_p"), False)

        @block.vector
        def _(vector):
            body(_Seq(vector, vs), True)

        if POOL_ON:
            @block.gpsimd
            def _(gpsimd):
                body(_Seq(gpsimd, ps), False)

        @block.scalar
        def _(scalar):
            # v-channel DMAs issued from the ACT queue (parallel w/ SP ring)
            xg = x[:, 1:2].rearrange("m c (s r) w -> s m c (r w)", r=32)
            scalar.dma_start(out=bv[:, 1:2, 0:GRID], in_=xg).then_inc(sv, 16)
            xh = x[:, 1:2, 32:256, :].rearrange("m c (s r) w -> s m c (r w)", r=32)
            scalar.dma_start(
                out=bv[0:112, 1:2, GRID:8704], in_=xh[:, :, :, 0:512]
            ).then_inc(sv, 16)
            scalar.dma_start(
                out=bv[112:128, 1:2, GRID:8704], in_=x[:, 1:2, 0:2, :]
            ).then_inc(sv, 16)

            A = _Seq(scalar, qs)

            def AW(name):
                A.wait(vs, M[name + "_v"])
                if POOL_ON:
                    A.wait(ps, M[name + "_p"])

            c3 = sq[:, 0:GRID].rearrange("p (r w) -> p r w", w=W)
            d3 = dc[:, 0:CH].rearrange("p (r w) -> p r w", w=W)
            z = zap[:, 0:1]

            AW("cont3")
            A.activation(c3[:, 0:30, 0:254], c3[:, 0:30, 0:254], AF.Abs,
                         bias=z, accum_out=ac[:, 0:1])
            A.activation(c3[:, 30:32, 0:254], c3[:, 30:32, 0:254], AF.Abs,
                         bias=z, accum_out=ac[:, 1:2])
            AW("sq4")
            A.activation(sq[:, 0:F33], sq[:, 0:F33], AF.Square, bias=z)
            AW("dk0")
            A.activation(d3[:, 0:16, 0:253], d3[:, 0:16, 0:253], AF.Abs,
                         bias=z, accum_out=ac[:, 2:3])
            AW("dk1")
            A.activation(d3[:, 0:14, 0:253], d3[:, 0:14, 0:253], AF.Abs,
                         bias=z, accum_out=ac[:, 3:4])
            A.activation(d3[:, 14:16, 0:253], d3[:, 14:16, 0:253], AF.Abs,
                         bias=z, accum_out=ac[:, 4:5])
            AW("sqv")
            A.activation(sq[:, 0:F33], sq[:, 0:F33], AF.Square, bias=z)
            AW("dpk0")
            A.activation(d3[:, 0:16, 0:254], d3[:, 0:16, 0:254], AF.Abs,
                         bias=z, accum_out=ac[:, 5:6])
            AW("dpk1")
            A.activation(d3[:, 0:13, 0:254], d3[:, 0:13, 0:254], AF.Abs,
                         bias=z, accum_out=ac[:, 6:7])
            A.activation(d3[:, 13:16, 0:254], d3[:, 13:16, 0:254], AF.Abs,
                         bias=z, accum_out=ac[:, 7:8])

        @block.sync
        def _(sync):
            for c, sem in ((0, su), (2, sp)):
                xg = x[:, c:c + 1].rearrange("m c (s r) w -> s m c (r w)", r=32)
                sync.dma_start(out=bv[:, c:c + 1, 0:GRID], in_=xg).then_inc(sem, 16)
                xh = x[:, c:c + 1, 32:256, :].rearrange(
                    "m c (s r) w -> s m c (r w)", r=32)
                sync.dma_start(
                    out=bv[0:112, c:c + 1, GRID:8704], in_=xh[:, :, :, 0:512]
                ).then_inc(sem, 16)
                sync.dma_start(
                    out=bv[112:128, c:c + 1, GRID:8704], in_=x[:, c:c + 1, 0:2, :]
                ).then_inc(sem, 16)
            sync.wait_ge(vs, M["end_v"])
            if POOL_ON:
                sync.wait_ge(ps, M["end_p"])
            sync.wait_ge(qs, 10)
            sync.dma_start(out=acc_d, in_=ac[:, :]).then_inc(so, 16)
            sync.wait_ge(so, 16)

    return nc


def _get_nc():
    if "nc" not in _CACHE:
        _CACHE["nc"] = _build_program()
    return _CACHE["nc"]


def combine(accs):
    """Host-side combine of per-core [128, NACC] partial-sum tensors."""
    cont_sum = 0.0
    pois_sum = 0.0
    bc_total = 0.0

    def msum(r, last):
        # r: [8 slabs, 16, 32 rows]; global rows 1..253 (last=30) or 1..254 (last=31)
        return (r[0, :, 1:].sum(-1) + r[1:7, :, :].sum((0, 2))
                + r[7, :, 0:last].sum(-1))

    for a in accs:
        A = np.asarray(a, dtype=np.float64).reshape(NSLAB, BPC, NACC)
        s_c = A[:, :, 0].sum(0) + A[0:7, :, 1].sum(0)
        s_d = A[:, :, 2].sum(0) + A[:, :, 3].sum(0) + A[0:7, :, 4].sum(0)
        s_dp = A[:, :, 5].sum(0) + A[:, :, 6].sum(0) + A[0:7, :, 7].sum(0)
        cont_sum += (s_c / (NCONT * DX)).sum()
        pois_sum += ((s_d + s_dp) / (NPOIS * 4.0 * DX)).sum()
        y0 = A[0, :, 8] + A[0, :, 9] + A[0, :, 10]
        yl = 2.0 * U0 * 253.0 - A[7, :, 11] + A[7, :, 12] + A[7, :, 13]
        x0 = (msum(A[:, :, 20:52], 30) + msum(A[:, :, 84:116], 31)
              + msum(A[:, :, 148:180], 31))
        xl = (msum(A[:, :, 52:84], 30) + msum(A[:, :, 116:148], 31)
              + msum(A[:, :, 180:212], 31))
        bc_total += (np.abs(y0) + np.abs(yl) + np.abs(x0) + np.abs(xl)).sum()
    nb = float(N_CORES * BPC)
    loss = LC * cont_sum / nb + (1.0 - LC - LB) * pois_sum / nb + LB * bc_total
    return np.array(loss, dtype=np.float32)


def run(gen_output, trace=False):
    gen_output = np.asarray(gen_output, dtype=np.float32)
    nc = _get_nc()
    in_maps = [
        {"x": np.ascontiguousarray(gen_output[c * BPC:(c + 1) * BPC])}
        for c in range(N_CORES)
    ]
    res = run_bass_kernel_spmd(nc, in_maps, core_ids=list(range(N_CORES)), trace=trace)
    _CACHE["last"] = res
    return combine([res.results[c]["acc"] for c in range(N_CORES)])


def kernel(gen_output):
    return run(gen_output)
